# revision 29
# baseline (speedup 1.0000x reference)
"""Trainium2 Bass kernel for nn_CosSimSpatTempConvNet.

Math (reference):
  merged[f,c,k] = conv_w[f,k] * spat_w[f,c]                  (rank-1 kernel)
  conved[b,f,t] = sum_{c,k} merged[f,c,k] * x[b,c,t+k]       (valid conv, Tout=T-K+1)
  norm_w[f]    = ||conv_w[f]|| * ||spat_w[f]||
  norm_in[b,t] = sqrt(sum_{c,k} x[b,c,t+k]^2)
  cos[b,f,t]   = conved * 64 / (norm_w[f] * norm_in[b,t])
  out[b]       = sum_f (mean_t |cos[b,f,t]| * weight[f] + bias[f])

Device strategy (8 cores, data-parallel over batch, 8 b per core):
  * Conv as fp8e4 DoubleRow TensorE matmuls: contraction 256 per
    instruction = 128 partitions (k2 in {0,1} x c) x 2 k-planes
    (j in {0,1}; k = 2*kq + k2 + 32*j).  16 PSUM-accumulated matmuls per
    512-wide time tile; sustained ~207-213 ns per 512-wide DR matmul is
    the practical fp8 roofline (157 TF/s), so the conv floor is ~215 us
    per core.  This kernel's job is to keep the PE stream at that floor.
  * Stationaries wdr[(k2,c), kq, j, f] = conv_w[f,2kq+k2+32j]*spat_w[f,c]
    host-precomputed (SwInterleave pairs), scaled by SW=128, clipped to
    +-240.  Odd batches keep their planes on partitions 64-127 and use
    the row-rolled stationary wdr_o (no cross-partition engine ops).
  * norm_in via fp8 DR matmuls too: ScalarE squares x into
    xsq8[(h,c), j, u] = (SQS*x[b_h, c, u+2048*j])^2 (j splits T in
    half), and a block-diagonal ones stationary produces all four
    (h, j) c-sum rows in one matmul -> 4 matmuls per pair instead of 8
    bf16 ones (saves ~14 us PE).  Window-64 sums run on the chunked
    [32, 320] layout (6 doubling shifts, DVE), then one ScalarE Rsqrt
    gives 1/(SQS*norm) directly (no DVE reciprocal).
  * epilogue per (b, t-tile): 1/norm row staged to DRAM and DMA-
    broadcast across 128 partitions; then ONE DVE op
    (|psum| via op0=abs_max vs 0) * pbs with accum_out row-sum.
    ScalarE is entirely out of the PSUM-eviction path.
  * software pipeline with one-batch lookahead: pair p+1's x-load,
    casts, squares, sq matmuls and norm chain are emitted at fixed
    wave boundaries inside pair p's convs, so every engine queue stays
    ahead of the PE and no pair-boundary stall occurs.  Pair 0 is
    loaded in four 1024-wide chunks with per-chunk casts/shifts so the
    first conv matmul issues ~6 us after start instead of ~23 us.
  * finish: S[f,b] sums -> one matmul with stationary
    g[f] = 64*SQS*weight/(4033*norm_w*SW*SX) contracting over f, + sum(bias).
"""

import contextlib
import ctypes
import sys
import types

import numpy as np

import concourse.bass as bass
import concourse.mybir as mybir
import concourse.tile as tile
from concourse.bass_utils import run_bass_kernel_spmd
from concourse.vector_clock import ScopedClock

F32 = mybir.dt.float32
BF16 = mybir.dt.bfloat16
F8 = mybir.dt.float8e4

B, CIN, T = 64, 64, 4096
F, K = 128, 64
TOUT = T - K + 1          # 4033
NCORES = 8
BLOC = B // NCORES        # 8 batches per core
KQ = 16                   # DoubleRow quad groups: k = 2*kq + k2 + 32*j
TS = 512                  # moving-operand tile (one fp32 PSUM bank)
NTILES = (TOUT + TS - 1) // TS      # 8 (last tile 449)
SCALE = 64.0              # sqrt(CIN*K)
SW = 128.0                # fp8 weight scale
SX = 16.0                 # fp8 x scale
SQS = 2.5                 # fp8 x-square scale: quantize (SQS*x)^2
NCH = 16                  # norm chunks per batch
CW = 256                  # chunk output width
CPAD = 320                # chunk width incl. 64-halo
T2 = T // 2               # 2048: j-plane width for xsq8
CH = 1024                 # pair-0 x-load chunk width
WAVES = [(0, 3), (3, 6), (6, 8)]

AF = mybir.ActivationFunctionType
ALU = mybir.AluOpType
DR = mybir.MatmulPerfMode.DoubleRowSwInterleave


# ---------------------------------------------------------------------------
# Container fixups: walrus here rejects >1 sem-wait on a Drain; TileContext's
# tail drain carries one wait per logical processor.  Chunk into single-wait
# drains.  Also recreate the (absent) antenv.axon_hooks NTFF profile hook so
# trace=True works when a test harness wants timings.
# ---------------------------------------------------------------------------

def _patched_drain_and_barrier(self, tick_clock, wait_clock):
    nc = self.nc
    drain_inst = nc.sync.drain()
    wait_clock.add_sem_waits(
        drain_inst.ins, ScopedClock({None: tick_clock.global_clock})
    )
    si = drain_inst.ins.sync_info
    waits = list(si.on_wait or []) if si else []
    if len(waits) > 1:
        si.on_wait = waits[:1]
        for w in waits[1:]:
            d2 = nc.sync.drain()
            si2 = d2.ins.sync_info
            if si2 is None:
                d2.ins.sync_info = mybir.SyncInfo(on_wait=[w], on_update=[])
            else:
                si2.on_wait = [w]
    nc.all_engine_barrier()
    assert self.sems is not None
    popped = nc._tile_sem_poison_stack.pop()
    assert popped is self._sem_poison
    nc.clear_and_free_semaphores(list(self.sems.allocated().values()))
    nc.all_engine_barrier()


def _install_ntff_hook():
    if "antenv.axon_hooks" in sys.modules:
        return
    try:
        lib = ctypes.CDLL("/opt/axon/libaxon_pjrt.so")
    except OSError:
        return
    if not hasattr(lib, "axon_start_nrt_profile"):
        return
    lib.axon_start_nrt_profile.argtypes = [
        ctypes.POINTER(ctypes.c_int64),
        ctypes.c_size_t,
    ]
    lib.axon_start_nrt_profile.restype = ctypes.c_int64
    lib.axon_stop_nrt_profile.argtypes = [ctypes.c_char_p]
    lib.axon_stop_nrt_profile.restype = ctypes.c_int64

    @contextlib.contextmanager
    def _hook(output_dir, device_ids):
        import jax

        jax.devices()
        if device_ids:
            ids = (ctypes.c_int64 * len(device_ids))(*device_ids)
            rc = lib.axon_start_nrt_profile(ids, len(device_ids))
        else:
            rc = lib.axon_start_nrt_profile(None, 0)
        if rc != 0:
            raise RuntimeError(f"axon_start_nrt_profile rc={rc}")
        try:
            yield
        finally:
            n = lib.axon_stop_nrt_profile(str(output_dir).encode())
            print(f"profile: {n} ntff file(s) in {output_dir}", file=sys.stderr)

    mod = types.ModuleType("antenv.axon_hooks")
    mod.get_axon_ntff_profile_hook = lambda: _hook
    mod.set_axon_ntff_profile_hook = lambda h: None
    import antenv

    antenv.axon_hooks = mod
    sys.modules["antenv.axon_hooks"] = mod


_ORIG_COMMIT = tile.TileContext._commit_instruction


def _commit_split_waits(self, inst, lazy_reg_writes=True):
    """walrus here allows only one sem-wait per instruction; move extras
    onto same-engine NOPs committed immediately before the instruction."""
    si = getattr(inst, "sync_info", None)
    if (
        si is not None
        and si.on_wait
        and len(si.on_wait) > 1
        and inst.engine != mybir.EngineType.Unassigned
    ):
        waits = list(si.on_wait)
        si.on_wait = waits[:1]
        for i, w in enumerate(waits[1:]):
            nop = mybir.InstNoOp(
                name=f"{inst.name}-wsplit{i}", ins=[], outs=[]
            )
            nop.engine = inst.engine
            nop.sync_info = mybir.SyncInfo(on_wait=[w], on_update=[])
            _ORIG_COMMIT(self, nop, lazy_reg_writes=False)
    return _ORIG_COMMIT(self, inst, lazy_reg_writes)


def _calibrate_sched_cost_model():
    """The tile scheduler derives its static per-engine instruction order
    from a CoreSim run.  CoreSim's full-p-state PE_CYCLE (0.4167 ns) is 2x
    faster than the sustained rate real TRN2 delivers for back-to-back
    matmuls (~207 ns per 512-wide fp8-DR matmul = the mid p-state), so the
    sim weaves producer work (casts/norms) too late relative to the conv
    waves and the real machine stalls.  Pin the scheduling model to the
    mid p-state, which matches every measured matmul class (fp8 DR 213 ns,
    bf16 427 ns).  Scheduling hint only -- semantics are unchanged.  Must
    run before the first cost-model use: the rust side caches hw specs in
    a process-wide OnceLock."""
    from concourse.hw_specs import TRN2Spec

    TRN2Spec.PE_CYCLE = TRN2Spec.PE_CYCLE_PSTATE_MID


def install_fixups():
    tile.TileContext._drain_and_barrier = _patched_drain_and_barrier
    tile.TileContext._commit_instruction = _commit_split_waits
    _calibrate_sched_cost_model()
    _install_ntff_hook()


def dedupe_ldweights(nc: bass.Bass) -> int:
    """Replace back-to-back InstLdweights that reload the identical
    stationary with same-sync NOPs.  walrus emits one LDWEIGHTS per
    non-f32 matmul; in a kq-outer wave the followers reload the same
    weights, costing ~120ns of PE each for nothing.  Weights persist in
    the array across matmuls, so a NOP carrying the original sync_info
    is semantically equivalent."""
    n = 0
    for fn in nc.m.functions:
        for bb in fn.blocks:
            last_key = None
            insts = bb.instructions
            for i, inst in enumerate(insts):
                if isinstance(inst, mybir.InstLdweights):
                    key = repr(inst.ins)
                    if key == last_key:
                        nop = mybir.InstNoOp(
                            name=f"{inst.name}-wdup", ins=[], outs=[]
                        )
                        nop.engine = inst.engine
                        nop.sync_info = inst.sync_info
                        insts[i] = nop
                        n += 1
                    else:
                        last_key = key
                elif isinstance(inst, mybir.InstMatmult):
                    if inst.ldweights is not False:
                        last_key = None
                elif getattr(inst, "engine", None) == getattr(
                    mybir.EngineType, "PE", None
                ):
                    last_key = None
    return n


# ---------------------------------------------------------------------------
# Device program (identical on all 8 cores; inputs differ per core)
# ---------------------------------------------------------------------------

def build_program() -> bass.Bass:
    install_fixups()
    nc = bass.Bass()

    xs_in = nc.dram_tensor("xs", [BLOC, CIN, T], F32, kind="ExternalInput")
    wdr_in = nc.dram_tensor("wdr", [128, KQ, 2 * F], F8, kind="ExternalInput")
    wdro_in = nc.dram_tensor("wdro", [128, KQ, 2 * F], F8, kind="ExternalInput")
    g_in = nc.dram_tensor("g", [F, 1], F32, kind="ExternalInput")
    bsum_in = nc.dram_tensor("bsum", [1, 1], F32, kind="ExternalInput")
    ones8_in = nc.dram_tensor("ones8", [128, 2 * F], F8, kind="ExternalInput")
    out_d = nc.dram_tensor("out", [1, BLOC], F32, kind="ExternalOutput")
    rdram = nc.dram_tensor("rdram", [BLOC, NCH, CW], BF16, kind="Internal")

    PS = bass.MemorySpace.PSUM

    with tile.TileContext(nc) as tc:
        with (
            tc.tile_pool(name="const", bufs=1) as constp,
            tc.tile_pool(name="xtp", bufs=2) as xtp,
            tc.tile_pool(name="sqp", bufs=2) as sqp,
            tc.tile_pool(name="s4p", bufs=2) as s4p,
            tc.tile_pool(name="xf8p", bufs=4) as xf8p,
            tc.tile_pool(name="xf8sp", bufs=2) as xf8sp,
            tc.tile_pool(name="slidep", bufs=3) as slidep,
            tc.tile_pool(name="rcp", bufs=2) as rcp,
            tc.tile_pool(name="pbsp", bufs=4) as pbsp,
            tc.tile_pool(name="scrp", bufs=3) as scrp,
            tc.tile_pool(name="accp", bufs=2) as accp,
            tc.tile_pool(name="pconv", bufs=6, space=PS) as pconv,
            tc.tile_pool(name="psq", bufs=2, space=PS) as psq,
        ):
            # constants -- triggered from the GpSimd queue so they don't
            # serialize behind the latency-critical pair-0 x chunks on Sync
            wdr = constp.tile([128, KQ, 2 * F], F8)
            nc.gpsimd.dma_start(wdr[:], wdr_in[:])
            wdr_o = constp.tile([128, KQ, 2 * F], F8)
            nc.gpsimd.dma_start(wdr_o[:], wdro_in[:])
            gsb = constp.tile([F, 1], F32)
            nc.gpsimd.dma_start(gsb[:], g_in[:])
            bsumsb = constp.tile([1, 1], F32)
            nc.gpsimd.dma_start(bsumsb[:], bsum_in[:])
            ones8 = constp.tile([128, 2 * F], F8)
            nc.gpsimd.dma_start(ones8[:], ones8_in[:])

            S = constp.tile([F, BLOC], F32)        # per-(f,b) |cos| sums

            # ---------------- pipeline stage helpers -----------------------

            def load_x(p):
                """Full-T DMA of x for batches (2p, 2p+1)."""
                xt2 = xtp.tile([128, T], F32, tag="xt", name=f"xt{p}")
                nc.sync.dma_start(xt2[:], xs_in[2 * p:2 * p + 2])
                return xt2

            def new_xf8(p, h, W=T, tag="xf8", pool=None):
                xf8 = (pool or xf8p).tile(
                    [128, 2, W], F8, tag=tag, name=f"xf8_{2 * p + h}_{W}"
                )
                nc.vector.memset(xf8[:, 0:2, W - 64:W], 0.0)
                return xf8

            def cast_planes(xf8, xt2, h, base, W, u0, u1):
                """All four (k2, j) fp8 planes of batch-half h for local
                u in [u0, u1): xf8[(k2,c), j, u] = SX*x[c, base+u+k2+32j].
                The k2=1 planes are written by CROSS-partition engine ops
                (read x rows on one 64-partition half, write the other),
                replacing the SBUF shift-DMA of the two-step scheme.  j=0
                planes go to ScalarE, j=1 to DVE."""
                xlo = 64 * h
                for k2 in (0, 1):
                    plo = 64 * h if k2 == 0 else 64 - 64 * h
                    for j in (0, 1):
                        off = k2 + 32 * j
                        hi = min(u1, W, T - base - off)
                        lo = min(u0, hi)
                        if lo >= hi:
                            continue
                        src = xt2[xlo:xlo + 64, base + lo + off:base + hi + off]
                        dst = xf8[plo:plo + 64, j, lo:hi]
                        if j == 0:
                            nc.scalar.activation(dst, src, AF.Copy, scale=SX)
                        else:
                            nc.vector.tensor_scalar_mul(dst, src, SX)

            def cast_half(xf8, xt2, h, t0, t1):
                """Steady-state cast for batch-half h (j planes on their
                own partitions), then one SBUF shift-DMA builds the k2=1
                planes on the other 64 partitions.  Cheaper on ScalarE
                than four cross-partition casts; fine off the cold path."""
                lo, hi = (0, 64) if h == 0 else (64, 128)
                nc.scalar.activation(
                    xf8[lo:hi, 0, t0:t1], xt2[lo:hi, t0:t1], AF.Copy, scale=SX
                )
                j0 = max(0, t0 - 32)
                nc.vector.tensor_scalar_mul(
                    xf8[lo:hi, 1, j0:t1 - 32], xt2[lo:hi, j0 + 32:t1], SX
                )
                olo = 64 - lo
                nc.sync.dma_start(
                    xf8[olo:olo + 64, 0:2, 0:T - 1],
                    xf8[lo:lo + 64, 0:2, 1:T],
                )

            def square_op(xsq8, xt2, j, u0, u1):
                """xsq8[:, j, u] = fp8((SQS * x[., u + 2048*j])^2)."""
                nc.scalar.activation(
                    xsq8[:, j, u0:u1],
                    xt2[:, T2 * j + u0:T2 * j + u1],
                    AF.Square,
                    scale=SQS,
                )

            def sq_group(p, xsq8):
                """4 fp8 DR matmuls: block-diag ones stationary gives rows
                (2h+j) = sum_c xsq8[(h,c), j, u] for all four (h, j) at
                once.  Copies rows 0:4 of each psum bank into the bf16
                staging tile s4[2h+j, chunk, 256]."""
                s4 = s4p.tile([4, 8, CW], BF16, tag="s4", name=f"s4_{p}")
                for ts in range(4):
                    pq = psq.tile([128, TS], F32, tag="pq", name=f"pq{p}_{ts}")
                    bi = nc.tensor.matmul(
                        pq[:, :],
                        ones8[:],
                        xsq8[:, 0:2, ts * TS:(ts + 1) * TS],
                        perf_mode=DR,
                    )
                    if ts > 0:
                        bi.ins.ldweights = False
                    nc.scalar.copy(s4[0:4, 2 * ts:2 * ts + 2, :], pq[0:4, :])
                return s4

            def norm_pair(p, s4):
                """Chunked sliding-window-64 sums + Rsqrt for a batch pair;
                1/(SQS*norm) rows to rdram[2p], rdram[2p+1].  sqc row
                16h + c holds chunk c (t in [256c, 256c+320)) of batch
                2p+h; chunk c lives in s4[2h + c//8, c%8, :]."""
                sqc = slidep.tile([2 * NCH, CPAD], BF16, tag="slide",
                                  name=f"sqc{p}")
                # positive filler first (chunk 15's halo is past T and only
                # feeds unused t0 > 4032); halo DMAs overwrite other rows
                nc.vector.memset(sqc[0:2 * NCH, CW:CPAD], 1.0)
                for h in range(2):
                    for j in range(2):
                        r0 = 16 * h + 8 * j
                        r = 2 * h + j
                        nc.sync.dma_start(
                            sqc[r0:r0 + 8, 0:CW], s4[r:r + 1, 0:8, :]
                        )
                        nc.sync.dma_start(
                            sqc[r0:r0 + 7, CW:CPAD], s4[r:r + 1, 1:8, 0:64]
                        )
                    # chunk 7's halo crosses into plane j=1's first 64
                    nc.sync.dma_start(
                        sqc[16 * h + 7:16 * h + 8, CW:CPAD],
                        s4[2 * h + 1:2 * h + 2, 0:1, 0:64],
                    )
                cur = sqc
                width = CPAD
                for sh in (1, 2, 4, 8, 16, 32):
                    width -= sh
                    nxt = slidep.tile([2 * NCH, CPAD], BF16, tag="slide",
                                      name=f"sl{p}_{sh}")
                    nc.vector.tensor_tensor(
                        nxt[:, 0:width], cur[:, 0:width],
                        cur[:, sh:sh + width], op=ALU.add,
                    )
                    cur = nxt
                assert width == CW + 1
                rec = rcp.tile([2 * NCH, CPAD], F32, tag="rec",
                               name=f"rec{p}")
                nc.vector.reciprocal(rec[:, 0:CW], cur[:, 0:CW])
                rc = rcp.tile([2 * NCH, CW], BF16, tag="rc", name=f"rc{p}")
                nc.scalar.activation(rc[:], rec[:, 0:CW], AF.Sqrt)
                nc.sync.dma_start(rdram[2 * p], rc[0:NCH, :])
                nc.sync.dma_start(rdram[2 * p + 1], rc[NCH:2 * NCH, :])

            def conv_batch(b, xf8, wtile, mid=None, late_pbs=(), xf8map=None,
                           waves=None):
                """Conv waves + fused cosine epilogue.  mid[wi] callables
                are emitted just before wave wi's matmuls: that is where
                next-pair prep work lands in each engine queue.  Waves in
                late_pbs defer their whole epilogue (pbs DMA + DVE evict)
                to just after the NEXT wave's mid callback -- needed when
                the rdram write this batch reads is emitted inside a mid
                callback (pair 0)."""
                mid = mid or {}
                acc = accp.tile([F, NTILES], F32, name=f"acc{b}")

                def emit_pbs(ts):
                    t = pbsp.tile(
                        [128, TS], BF16, tag="pbs", name=f"pbs{b}_{ts}"
                    )
                    nc.sync.dma_start(
                        t[:],
                        rdram[b, 2 * ts:2 * ts + 2, 0:CW]
                        .partition_broadcast(128),
                    )
                    return t

                def emit_evict(ts, pc, pb):
                    nt = min(TS, TOUT - ts * TS)
                    # DVE may not apply abs in scalar_tensor_tensor: take
                    # |conv| on ScalarE (PSUM->SBUF), then multiply by
                    # 1/norm + row-sum into acc[:, ts] on DVE
                    scr = scrp.tile(
                        [F, TS], BF16, tag="scr", name=f"scr{b}_{ts}"
                    )
                    nc.scalar.activation(
                        scr[:, 0:nt], pc[:, 0:nt], AF.Abs
                    )
                    nc.vector.scalar_tensor_tensor(
                        scr[:, 0:nt],
                        scr[:, 0:nt],
                        1.0,
                        pb[:, 0:nt],
                        op0=ALU.mult,
                        op1=ALU.mult,
                        accum_out=acc[:, ts:ts + 1],
                    )

                deferred = []
                for wi, (w0, w1) in enumerate(waves or WAVES):
                    if wi in mid:
                        mid[wi]()
                    for ts, pc in deferred:
                        emit_evict(ts, pc, emit_pbs(ts))
                    deferred = []
                    wtiles = list(range(w0, w1))
                    pcs = {}
                    for ts in wtiles:
                        pcs[ts] = pconv.tile(
                            [F, TS], F32, name=f"pc_{b}_{ts}", tag="pc"
                        )
                    # prefetch 1/norm broadcast rows while the wave runs
                    pbs = {}
                    if wi not in late_pbs:
                        for ts in wtiles:
                            pbs[ts] = emit_pbs(ts)
                    # kq-outer: one stationary serves len(wtiles) matmuls;
                    # followers skip the redundant LDWEIGHTS
                    for kq in range(KQ):
                        for i, ts in enumerate(wtiles):
                            t0 = ts * TS
                            nt = min(TS, TOUT - t0)
                            nt_mm = nt + (nt & 1)
                            mvt, base = (
                                xf8map(ts) if xf8map is not None else (xf8, 0)
                            )
                            u0 = t0 - base + 2 * kq
                            bi = nc.tensor.matmul(
                                pcs[ts][:, 0:nt_mm],
                                wtile[:, kq, :],
                                mvt[:, 0:2, u0:u0 + nt_mm],
                                start=(kq == 0),
                                stop=(kq == KQ - 1),
                                perf_mode=DR,
                            )
                            if i > 0:
                                bi.ins.ldweights = False
                    if wi in late_pbs:
                        deferred = [(ts, pcs[ts]) for ts in wtiles]
                    else:
                        for ts in wtiles:
                            emit_evict(ts, pcs[ts], pbs[ts])
                for ts, pc in deferred:
                    emit_evict(ts, pc, emit_pbs(ts))
                nc.vector.reduce_sum(
                    S[:, b:b + 1], acc[:], axis=mybir.AxisListType.X
                )

            # ---------------- pair 0: chunked cold start --------------------
            # x arrives in three chunks; batch-A's moving operand is split
            # into L/M/R tiles so conv wave 0 (tiles 0-1) waits only on
            # the four L casts built from the small first chunk: the
            # framework's counting-semaphore deps would otherwise make
            # wave 0 wait on every writer of a single whole-T tile.
            WL, WM, MBASE, WR, RBASE = 1088, 1600, 1024, 1536, 2560
            C0, C1 = 1152, 2688
            xt2_0 = xtp.tile([128, T], F32, tag="xt", name="xt0c")
            nc.sync.dma_start(xt2_0[:, 0:C0], xs_in[0:2, :, 0:C0])
            nc.sync.dma_start(xt2_0[:, C0:C1], xs_in[0:2, :, C0:C1])
            nc.sync.dma_start(xt2_0[:, C1:T], xs_in[0:2, :, C1:T])
            xsq8_0 = sqp.tile([128, 2, T2], F8, tag="xsq", name="xsq0")
            xf8_l = xf8sp.tile([128, 2, WL], F8, tag="xf8s", name="xf8L")
            cast_planes(xf8_l, xt2_0, 0, 0, WL, 0, WL)
            xf8_m = xf8sp.tile([128, 2, WM], F8, tag="xf8s", name="xf8M")
            cast_planes(xf8_m, xt2_0, 0, MBASE, WM, 0, WM)
            square_op(xsq8_0, xt2_0, 0, 0, T2)
            xf8_r = new_xf8(0, 0, W=WR, tag="xf8s", pool=xf8sp)
            cast_planes(xf8_r, xt2_0, 0, RBASE, WR, 0, WR)
            square_op(xsq8_0, xt2_0, 1, 0, T2)

            def map0(ts):
                if ts < 2:
                    return (xf8_l, 0)
                if ts < 5:
                    return (xf8_m, MBASE)
                return (xf8_r, RBASE)

            xf8_b = new_xf8(0, 1)

            st = {"p": 0, "xt2": None, "xsq8": None, "s4": None,
                  "a": None, "b": None}

            def prep0_mid():
                # pair 0's norm chain + batch-B casts, emitted between
                # batch 0's conv waves (ScalarE/DVE/PE all idle there)
                s4 = sq_group(0, xsq8_0)
                cast_half(xf8_b, xt2_0, 1, 0, T)
                norm_pair(0, s4)

            def mk_loadx(q):
                def f():
                    st["xt2"] = load_x(q)
                return f

            def mk_casta(q):
                def f():
                    st["a"] = new_xf8(q, 0)
                    cast_half(st["a"], st["xt2"], 0, 0, T)
                return f

            def mk_square(q):
                def f():
                    st["xsq8"] = sqp.tile(
                        [128, 2, T2], F8, tag="xsq", name=f"xsq{q}"
                    )
                    square_op(st["xsq8"], st["xt2"], 0, 0, T2)
                    square_op(st["xsq8"], st["xt2"], 1, 0, T2)
                return f

            def mk_sqnorm(q):
                def f():
                    s4 = sq_group(q, st["xsq8"])
                    norm_pair(q, s4)
                return f

            def mk_castb(q):
                def f():
                    st["b"] = new_xf8(q, 1)
                    cast_half(st["b"], st["xt2"], 1, 0, T)
                return f

            # pair 0 convs (prep for pair 1 embedded); batch 0's wave-0
            # pbs DMA must be emitted after prep0_mid's rdram write.
            # load_x(1) sits at mid[1] so its 2 MB doesn't compete with
            # pair 0's halves for DMA bandwidth.
            conv_batch(
                0, None, wdr, xf8map=map0, waves=[(0, 2), (2, 5), (5, 8)],
                mid={1: lambda: (prep0_mid(), mk_loadx(1)())}, late_pbs=(0,),
            )
            conv_batch(
                1, xf8_b, wdr_o,
                mid={1: lambda: (mk_square(1)(), mk_casta(1)()),
                     2: lambda: (mk_sqnorm(1)(), mk_castb(1)())},
            )

            # steady pairs 1..3 (prep for pair q=p+1 embedded)
            for p in range(1, 4):
                xf8_a, xf8_b = st["a"], st["b"]
                q = p + 1
                if q < 4:
                    mid_a = {0: mk_loadx(q), 1: mk_casta(q), 2: mk_square(q)}
                    mid_b = {1: mk_sqnorm(q), 2: mk_castb(q)}
                else:
                    mid_a = None
                    mid_b = None
                conv_batch(2 * p, xf8_a, wdr, mid=mid_a)
                conv_batch(2 * p + 1, xf8_b, wdr_o, mid=mid_b)

            # ---- finish: out[b] = sum_f g[f]*S[f,b] + sum_f bias[f] ------
            pf = pconv.tile([1, BLOC], F32, name="pf", tag="pc")
            nc.tensor.matmul(pf[:], gsb[:], S[:])
            out_sb = constp.tile([1, BLOC], F32)
            nc.scalar.add(out_sb[:], pf[:], bsumsb[0:1, 0:1])
            nc.sync.dma_start(out_d[:], out_sb[:])

    ndup = dedupe_ldweights(nc)
    print(f"dedupe_ldweights: replaced {ndup}", file=sys.stderr)
    return nc


_PROGRAM: bass.Bass | None = None


def _get_program() -> bass.Bass:
    global _PROGRAM
    if _PROGRAM is None:
        _PROGRAM = build_program()
    return _PROGRAM


# ---------------------------------------------------------------------------
# Host entry point
# ---------------------------------------------------------------------------

def host_params(conv_weights, spat_weights, weight, bias):
    """Tiny host-side precomputation of stationaries and scalars."""
    conv = np.asarray(conv_weights, dtype=np.float64)
    spat = np.asarray(spat_weights, dtype=np.float64)
    w = np.asarray(weight, dtype=np.float64)
    bb = np.asarray(bias, dtype=np.float64)

    # prod[k, c, f] = conv[f, k] * spat[f, c]; DoubleRow packing
    # k = 32*j + 2*kq + k2  ->  wdr[(k2,c), kq, j, f]
    prod = np.einsum("fk,fc->kcf", conv, spat) * SW    # [K, C, F]
    P5 = prod.reshape(2, KQ, 2, CIN, F)                # [j, kq, k2, c, f]
    wdr = np.ascontiguousarray(P5.transpose(2, 3, 1, 0, 4)).reshape(
        128, KQ, 2, F
    )
    # SwInterleave layout: stored[p, kq, 2*(127-f) + j] = W[p, kq, j, f]
    wdr = np.ascontiguousarray(
        wdr[:, :, :, ::-1].transpose(0, 1, 3, 2)
    ).reshape(128, KQ, 2 * F)
    wdr = np.clip(wdr, -240.0, 240.0)
    f8np = mybir.dt.np(F8)
    wdr = wdr.astype(np.float32)
    wdro = np.roll(wdr, 64, axis=0)
    wdr = wdr.astype(f8np)
    wdro = wdro.astype(f8np)

    # block-diag ones stationary for the norm c-sums: output row
    # f = 2h + j sums plane j over batch-half h's partitions
    W5 = np.zeros((128, 2, F), np.float32)             # [p, j, f]
    W5[0:64, 0, 0] = 1.0
    W5[0:64, 1, 1] = 1.0
    W5[64:128, 0, 2] = 1.0
    W5[64:128, 1, 3] = 1.0
    ones8 = np.ascontiguousarray(
        W5[:, :, ::-1].transpose(0, 2, 1)
    ).reshape(128, 2 * F).astype(f8np)

    norm_w = np.sqrt((spat * spat).sum(1) * (conv * conv).sum(1))  # [F]
    g = (SCALE * SQS / (TOUT * norm_w * SW * SX) * w).astype(
        np.float32
    ).reshape(F, 1)
    bsum = np.array([[bb.sum()]], dtype=np.float32)
    return wdr, wdro, ones8, g, bsum


def make_in_maps(x, conv_weights, spat_weights, weight, bias):
    x = np.ascontiguousarray(np.asarray(x, dtype=np.float32))
    wdr, wdro, ones8, g, bsum = host_params(
        conv_weights, spat_weights, weight, bias
    )
    in_maps = []
    for c in range(NCORES):
        in_maps.append(
            {
                "xs": np.ascontiguousarray(x[c * BLOC:(c + 1) * BLOC]),
                "wdr": wdr,
                "wdro": wdro,
                "g": g,
                "bsum": bsum,
                "ones8": ones8,
            }
        )
    return in_maps


def kernel(x, conv_weights, spat_weights, weight, bias):
    in_maps = make_in_maps(x, conv_weights, spat_weights, weight, bias)
    nc = _get_program()
    res = run_bass_kernel_spmd(nc, in_maps, core_ids=list(range(NCORES)))
    out = np.concatenate(
        [res.results[c]["out"].reshape(BLOC) for c in range(NCORES)]
    )
    return out.astype(np.float32)


# revision 35
# speedup vs baseline: 1.0021x; 1.0021x over previous
"""Trainium2 Bass kernel for nn_CosSimSpatTempConvNet.

Math (reference):
  merged[f,c,k] = conv_w[f,k] * spat_w[f,c]                  (rank-1 kernel)
  conved[b,f,t] = sum_{c,k} merged[f,c,k] * x[b,c,t+k]       (valid conv, Tout=T-K+1)
  norm_w[f]    = ||conv_w[f]|| * ||spat_w[f]||
  norm_in[b,t] = sqrt(sum_{c,k} x[b,c,t+k]^2)
  cos[b,f,t]   = conved * 64 / (norm_w[f] * norm_in[b,t])
  out[b]       = sum_f (mean_t |cos[b,f,t]| * weight[f] + bias[f])

Device strategy (8 cores, data-parallel over batch, 8 b per core):
  * Conv as fp8e4 DoubleRow TensorE matmuls: contraction 256 per
    instruction = 128 partitions (k2 in {0,1} x c) x 2 k-planes
    (j in {0,1}; k = 2*kq + k2 + 32*j).  16 PSUM-accumulated matmuls per
    512-wide time tile; sustained ~207-213 ns per 512-wide DR matmul is
    the practical fp8 roofline (157 TF/s), so the conv floor is ~215 us
    per core.  This kernel's job is to keep the PE stream at that floor.
  * Stationaries wdr[(k2,c), kq, j, f] = conv_w[f,2kq+k2+32j]*spat_w[f,c]
    host-precomputed (SwInterleave pairs), scaled by SW=128, clipped to
    +-240.  Odd batches keep their planes on partitions 64-127 and use
    the row-rolled stationary wdr_o (no cross-partition engine ops).
  * norm_in via fp8 DR matmuls too: ScalarE squares x into
    xsq8[(h,c), j, u] = (SQS*x[b_h, c, u+2048*j])^2 (j splits T in
    half), and a block-diagonal ones stationary produces all four
    (h, j) c-sum rows in one matmul -> 4 matmuls per pair instead of 8
    bf16 ones (saves ~14 us PE).  Window-64 sums run on the chunked
    [32, 320] layout (6 doubling shifts, DVE), then one ScalarE Rsqrt
    gives 1/(SQS*norm) directly (no DVE reciprocal).
  * epilogue per (b, t-tile): 1/norm row staged to DRAM and DMA-
    broadcast across 128 partitions; then ONE DVE op
    (|psum| via op0=abs_max vs 0) * pbs with accum_out row-sum.
    ScalarE is entirely out of the PSUM-eviction path.
  * software pipeline with one-batch lookahead: pair p+1's x-load,
    casts, squares, sq matmuls and norm chain are emitted at fixed
    wave boundaries inside pair p's convs, so every engine queue stays
    ahead of the PE and no pair-boundary stall occurs.  Pair 0 is
    loaded in four 1024-wide chunks with per-chunk casts/shifts so the
    first conv matmul issues ~6 us after start instead of ~23 us.
  * finish: S[f,b] sums -> one matmul with stationary
    g[f] = 64*SQS*weight/(4033*norm_w*SW*SX) contracting over f, + sum(bias).
"""

import contextlib
import ctypes
import sys
import types

import numpy as np

import concourse.bass as bass
import concourse.mybir as mybir
import concourse.tile as tile
from concourse.bass_utils import run_bass_kernel_spmd
from concourse.vector_clock import ScopedClock

F32 = mybir.dt.float32
BF16 = mybir.dt.bfloat16
F8 = mybir.dt.float8e4

B, CIN, T = 64, 64, 4096
F, K = 128, 64
TOUT = T - K + 1          # 4033
NCORES = 8
BLOC = B // NCORES        # 8 batches per core
KQ = 16                   # DoubleRow quad groups: k = 2*kq + k2 + 32*j
TS = 512                  # moving-operand tile (one fp32 PSUM bank)
NTILES = (TOUT + TS - 1) // TS      # 8 (last tile 449)
SCALE = 64.0              # sqrt(CIN*K)
SW = 128.0                # fp8 weight scale
SX = 16.0                 # fp8 x scale
SQS = 2.5                 # fp8 x-square scale: quantize (SQS*x)^2
NCH = 16                  # norm chunks per batch
CW = 256                  # chunk output width
CPAD = 320                # chunk width incl. 64-halo
T2 = T // 2               # 2048: j-plane width for xsq8
CH = 1024                 # pair-0 x-load chunk width
WAVES = [(0, 3), (3, 6), (6, 8)]

AF = mybir.ActivationFunctionType
ALU = mybir.AluOpType
DR = mybir.MatmulPerfMode.DoubleRowSwInterleave


# ---------------------------------------------------------------------------
# Container fixups: walrus here rejects >1 sem-wait on a Drain; TileContext's
# tail drain carries one wait per logical processor.  Chunk into single-wait
# drains.  Also recreate the (absent) antenv.axon_hooks NTFF profile hook so
# trace=True works when a test harness wants timings.
# ---------------------------------------------------------------------------

def _patched_drain_and_barrier(self, tick_clock, wait_clock):
    nc = self.nc
    drain_inst = nc.sync.drain()
    wait_clock.add_sem_waits(
        drain_inst.ins, ScopedClock({None: tick_clock.global_clock})
    )
    si = drain_inst.ins.sync_info
    waits = list(si.on_wait or []) if si else []
    if len(waits) > 1:
        si.on_wait = waits[:1]
        for w in waits[1:]:
            d2 = nc.sync.drain()
            si2 = d2.ins.sync_info
            if si2 is None:
                d2.ins.sync_info = mybir.SyncInfo(on_wait=[w], on_update=[])
            else:
                si2.on_wait = [w]
    nc.all_engine_barrier()
    assert self.sems is not None
    popped = nc._tile_sem_poison_stack.pop()
    assert popped is self._sem_poison
    nc.clear_and_free_semaphores(list(self.sems.allocated().values()))
    nc.all_engine_barrier()


def _install_ntff_hook():
    if "antenv.axon_hooks" in sys.modules:
        return
    try:
        lib = ctypes.CDLL("/opt/axon/libaxon_pjrt.so")
    except OSError:
        return
    if not hasattr(lib, "axon_start_nrt_profile"):
        return
    lib.axon_start_nrt_profile.argtypes = [
        ctypes.POINTER(ctypes.c_int64),
        ctypes.c_size_t,
    ]
    lib.axon_start_nrt_profile.restype = ctypes.c_int64
    lib.axon_stop_nrt_profile.argtypes = [ctypes.c_char_p]
    lib.axon_stop_nrt_profile.restype = ctypes.c_int64

    @contextlib.contextmanager
    def _hook(output_dir, device_ids):
        import jax

        jax.devices()
        if device_ids:
            ids = (ctypes.c_int64 * len(device_ids))(*device_ids)
            rc = lib.axon_start_nrt_profile(ids, len(device_ids))
        else:
            rc = lib.axon_start_nrt_profile(None, 0)
        if rc != 0:
            raise RuntimeError(f"axon_start_nrt_profile rc={rc}")
        try:
            yield
        finally:
            n = lib.axon_stop_nrt_profile(str(output_dir).encode())
            print(f"profile: {n} ntff file(s) in {output_dir}", file=sys.stderr)

    mod = types.ModuleType("antenv.axon_hooks")
    mod.get_axon_ntff_profile_hook = lambda: _hook
    mod.set_axon_ntff_profile_hook = lambda h: None
    import antenv

    antenv.axon_hooks = mod
    sys.modules["antenv.axon_hooks"] = mod


_ORIG_COMMIT = tile.TileContext._commit_instruction


def _commit_split_waits(self, inst, lazy_reg_writes=True):
    """walrus here allows only one sem-wait per instruction; move extras
    onto same-engine NOPs committed immediately before the instruction."""
    si = getattr(inst, "sync_info", None)
    if (
        si is not None
        and si.on_wait
        and len(si.on_wait) > 1
        and inst.engine != mybir.EngineType.Unassigned
    ):
        waits = list(si.on_wait)
        si.on_wait = waits[:1]
        for i, w in enumerate(waits[1:]):
            nop = mybir.InstNoOp(
                name=f"{inst.name}-wsplit{i}", ins=[], outs=[]
            )
            nop.engine = inst.engine
            nop.sync_info = mybir.SyncInfo(on_wait=[w], on_update=[])
            _ORIG_COMMIT(self, nop, lazy_reg_writes=False)
    return _ORIG_COMMIT(self, inst, lazy_reg_writes)


def _calibrate_sched_cost_model():
    """The tile scheduler derives its static per-engine instruction order
    from a CoreSim run.  CoreSim's full-p-state PE_CYCLE (0.4167 ns) is 2x
    faster than the sustained rate real TRN2 delivers for back-to-back
    matmuls (~207 ns per 512-wide fp8-DR matmul = the mid p-state), so the
    sim weaves producer work (casts/norms) too late relative to the conv
    waves and the real machine stalls.  Pin the scheduling model to the
    mid p-state, which matches every measured matmul class (fp8 DR 213 ns,
    bf16 427 ns).  Scheduling hint only -- semantics are unchanged.  Must
    run before the first cost-model use: the rust side caches hw specs in
    a process-wide OnceLock."""
    from concourse.hw_specs import TRN2Spec

    TRN2Spec.PE_CYCLE = TRN2Spec.PE_CYCLE_PSTATE_MID


def install_fixups():
    tile.TileContext._drain_and_barrier = _patched_drain_and_barrier
    tile.TileContext._commit_instruction = _commit_split_waits
    _calibrate_sched_cost_model()
    _install_ntff_hook()


def dedupe_ldweights(nc: bass.Bass) -> int:
    """Replace back-to-back InstLdweights that reload the identical
    stationary with same-sync NOPs.  walrus emits one LDWEIGHTS per
    non-f32 matmul; in a kq-outer wave the followers reload the same
    weights, costing ~120ns of PE each for nothing.  Weights persist in
    the array across matmuls, so a NOP carrying the original sync_info
    is semantically equivalent."""
    n = 0
    for fn in nc.m.functions:
        for bb in fn.blocks:
            last_key = None
            insts = bb.instructions
            for i, inst in enumerate(insts):
                if isinstance(inst, mybir.InstLdweights):
                    key = repr(inst.ins)
                    if key == last_key:
                        nop = mybir.InstNoOp(
                            name=f"{inst.name}-wdup", ins=[], outs=[]
                        )
                        nop.engine = inst.engine
                        nop.sync_info = inst.sync_info
                        insts[i] = nop
                        n += 1
                    else:
                        last_key = key
                elif isinstance(inst, mybir.InstMatmult):
                    if inst.ldweights is not False:
                        last_key = None
                elif getattr(inst, "engine", None) == getattr(
                    mybir.EngineType, "PE", None
                ):
                    last_key = None
    return n


# ---------------------------------------------------------------------------
# Device program (identical on all 8 cores; inputs differ per core)
# ---------------------------------------------------------------------------

def build_program() -> bass.Bass:
    install_fixups()
    nc = bass.Bass()

    xs_in = nc.dram_tensor("xs", [BLOC, CIN, T], F32, kind="ExternalInput")
    wdr_in = nc.dram_tensor("wdr", [128, KQ, 2 * F], F8, kind="ExternalInput")
    wdro_in = nc.dram_tensor("wdro", [128, KQ, 2 * F], F8, kind="ExternalInput")
    g_in = nc.dram_tensor("g", [F, 1], F32, kind="ExternalInput")
    bsum_in = nc.dram_tensor("bsum", [1, 1], F32, kind="ExternalInput")
    ones8_in = nc.dram_tensor("ones8", [128, 2 * F], F8, kind="ExternalInput")
    out_d = nc.dram_tensor("out", [1, BLOC], F32, kind="ExternalOutput")
    rdram = nc.dram_tensor("rdram", [BLOC, NCH, CW], BF16, kind="Internal")

    PS = bass.MemorySpace.PSUM

    with tile.TileContext(nc) as tc:
        with (
            tc.tile_pool(name="const", bufs=1) as constp,
            tc.tile_pool(name="xtp", bufs=2) as xtp,
            tc.tile_pool(name="sqp", bufs=2) as sqp,
            tc.tile_pool(name="s4p", bufs=2) as s4p,
            tc.tile_pool(name="xf8p", bufs=4) as xf8p,
            tc.tile_pool(name="xf8sp", bufs=2) as xf8sp,
            tc.tile_pool(name="slidep", bufs=3) as slidep,
            tc.tile_pool(name="rcp", bufs=2) as rcp,
            tc.tile_pool(name="pbsp", bufs=4) as pbsp,
            tc.tile_pool(name="scrp", bufs=3) as scrp,
            tc.tile_pool(name="accp", bufs=2) as accp,
            tc.tile_pool(name="pconv", bufs=6, space=PS) as pconv,
            tc.tile_pool(name="psq", bufs=2, space=PS) as psq,
        ):
            # constants -- triggered from the GpSimd queue so they don't
            # serialize behind the latency-critical pair-0 x chunks on Sync
            wdr = constp.tile([128, KQ, 2 * F], F8)
            nc.gpsimd.dma_start(wdr[:], wdr_in[:])
            wdr_o = constp.tile([128, KQ, 2 * F], F8)
            nc.gpsimd.dma_start(wdr_o[:], wdro_in[:])
            gsb = constp.tile([F, 1], F32)
            nc.gpsimd.dma_start(gsb[:], g_in[:])
            bsumsb = constp.tile([1, 1], F32)
            nc.gpsimd.dma_start(bsumsb[:], bsum_in[:])
            ones8 = constp.tile([128, 2 * F], F8)
            nc.gpsimd.dma_start(ones8[:], ones8_in[:])

            S = constp.tile([F, BLOC], F32)        # per-(f,b) |cos| sums

            # ---------------- pipeline stage helpers -----------------------

            def load_x(p):
                """Full-T DMA of x for batches (2p, 2p+1).  Triggered from
                the (otherwise idle) GpSimd queue: the Sync queue's ~600ns
                per-trigger serialization must stay reserved for the
                latency-critical per-wave pbs broadcasts."""
                xt2 = xtp.tile([128, T], F32, tag="xt", name=f"xt{p}")
                nc.gpsimd.dma_start(xt2[:], xs_in[2 * p:2 * p + 2])
                return xt2

            def new_xf8(p, h, W=T, tag="xf8", pool=None):
                xf8 = (pool or xf8p).tile(
                    [128, 2, W], F8, tag=tag, name=f"xf8_{2 * p + h}_{W}"
                )
                nc.vector.memset(xf8[:, 0:2, W - 64:W], 0.0)
                return xf8

            def cast_planes(xf8, xt2, h, base, W, u0, u1):
                """All four (k2, j) fp8 planes of batch-half h for local
                u in [u0, u1): xf8[(k2,c), j, u] = SX*x[c, base+u+k2+32j].
                The k2=1 planes are written by CROSS-partition engine ops
                (read x rows on one 64-partition half, write the other),
                replacing the SBUF shift-DMA of the two-step scheme.  j=0
                planes go to ScalarE, j=1 to DVE."""
                xlo = 64 * h
                for k2 in (0, 1):
                    plo = 64 * h if k2 == 0 else 64 - 64 * h
                    for j in (0, 1):
                        off = k2 + 32 * j
                        hi = min(u1, W, T - base - off)
                        lo = min(u0, hi)
                        if lo >= hi:
                            continue
                        src = xt2[xlo:xlo + 64, base + lo + off:base + hi + off]
                        dst = xf8[plo:plo + 64, j, lo:hi]
                        if j == 0:
                            nc.scalar.activation(dst, src, AF.Copy, scale=SX)
                        else:
                            nc.vector.tensor_scalar_mul(dst, src, SX)

            def cast_half(xf8, xt2, h, t0, t1):
                """Steady-state cast for batch-half h (j planes on their
                own partitions), then one SBUF shift-DMA builds the k2=1
                planes on the other 64 partitions.  Cheaper on ScalarE
                than four cross-partition casts; fine off the cold path."""
                lo, hi = (0, 64) if h == 0 else (64, 128)
                nc.scalar.activation(
                    xf8[lo:hi, 0, t0:t1], xt2[lo:hi, t0:t1], AF.Copy, scale=SX
                )
                j0 = max(0, t0 - 32)
                nc.vector.tensor_scalar_mul(
                    xf8[lo:hi, 1, j0:t1 - 32], xt2[lo:hi, j0 + 32:t1], SX
                )
                olo = 64 - lo
                nc.gpsimd.dma_start(
                    xf8[olo:olo + 64, 0:2, 0:T - 1],
                    xf8[lo:lo + 64, 0:2, 1:T],
                )

            def square_op(xsq8, xt2, j, u0, u1):
                """xsq8[:, j, u] = fp8((SQS * x[., u + 2048*j])^2)."""
                nc.scalar.activation(
                    xsq8[:, j, u0:u1],
                    xt2[:, T2 * j + u0:T2 * j + u1],
                    AF.Square,
                    scale=SQS,
                )

            def sq_group(p, xsq8):
                """4 fp8 DR matmuls: block-diag ones stationary gives rows
                (2h+j) = sum_c xsq8[(h,c), j, u] for all four (h, j) at
                once.  Copies rows 0:4 of each psum bank into the bf16
                staging tile s4[2h+j, chunk, 256]."""
                s4 = s4p.tile([4, 8, CW], BF16, tag="s4", name=f"s4_{p}")
                for ts in range(4):
                    pq = psq.tile([128, TS], F32, tag="pq", name=f"pq{p}_{ts}")
                    bi = nc.tensor.matmul(
                        pq[:, :],
                        ones8[:],
                        xsq8[:, 0:2, ts * TS:(ts + 1) * TS],
                        perf_mode=DR,
                    )
                    if ts > 0:
                        bi.ins.ldweights = False
                    nc.scalar.copy(s4[0:4, 2 * ts:2 * ts + 2, :], pq[0:4, :])
                return s4

            def norm_pair(p, s4):
                """Chunked sliding-window-64 sums + Rsqrt for a batch pair;
                1/(SQS*norm) rows to rdram[2p], rdram[2p+1].  sqc row
                16h + c holds chunk c (t in [256c, 256c+320)) of batch
                2p+h; chunk c lives in s4[2h + c//8, c%8, :]."""
                sqc = slidep.tile([2 * NCH, CPAD], BF16, tag="slide",
                                  name=f"sqc{p}")
                # positive filler first (chunk 15's halo is past T and only
                # feeds unused t0 > 4032); halo DMAs overwrite other rows
                nc.vector.memset(sqc[0:2 * NCH, CW:CPAD], 1.0)
                for h in range(2):
                    for j in range(2):
                        r0 = 16 * h + 8 * j
                        r = 2 * h + j
                        nc.gpsimd.dma_start(
                            sqc[r0:r0 + 8, 0:CW], s4[r:r + 1, 0:8, :]
                        )
                        nc.gpsimd.dma_start(
                            sqc[r0:r0 + 7, CW:CPAD], s4[r:r + 1, 1:8, 0:64]
                        )
                    # chunk 7's halo crosses into plane j=1's first 64
                    nc.gpsimd.dma_start(
                        sqc[16 * h + 7:16 * h + 8, CW:CPAD],
                        s4[2 * h + 1:2 * h + 2, 0:1, 0:64],
                    )
                cur = sqc
                width = CPAD
                for sh in (1, 2, 4, 8, 16, 32):
                    width -= sh
                    nxt = slidep.tile([2 * NCH, CPAD], BF16, tag="slide",
                                      name=f"sl{p}_{sh}")
                    nc.vector.tensor_tensor(
                        nxt[:, 0:width], cur[:, 0:width],
                        cur[:, sh:sh + width], op=ALU.add,
                    )
                    cur = nxt
                assert width == CW + 1
                rec = rcp.tile([2 * NCH, CPAD], F32, tag="rec",
                               name=f"rec{p}")
                nc.vector.reciprocal(rec[:, 0:CW], cur[:, 0:CW])
                rc = rcp.tile([2 * NCH, CW], BF16, tag="rc", name=f"rc{p}")
                nc.scalar.activation(rc[:], rec[:, 0:CW], AF.Sqrt)
                nc.gpsimd.dma_start(rdram[2 * p], rc[0:NCH, :])
                nc.gpsimd.dma_start(rdram[2 * p + 1], rc[NCH:2 * NCH, :])

            def conv_batch(b, xf8, wtile, mid=None, late_pbs=(), xf8map=None,
                           waves=None):
                """Conv waves + fused cosine epilogue.  mid[wi] callables
                are emitted just before wave wi's matmuls: that is where
                next-pair prep work lands in each engine queue.  Waves in
                late_pbs defer their whole epilogue (pbs DMA + DVE evict)
                to just after the NEXT wave's mid callback -- needed when
                the rdram write this batch reads is emitted inside a mid
                callback (pair 0)."""
                mid = mid or {}
                acc = accp.tile([F, NTILES], F32, name=f"acc{b}")

                def emit_pbs(w0, w1):
                    """ONE broadcast DMA per wave (not per tile): each
                    Sync-queue trigger costs ~600ns of serial descriptor
                    programming, which was delaying latency-critical DMAs
                    behind per-tile prefetches."""
                    t = pbsp.tile(
                        [128, 3, TS], BF16, tag="pbs", name=f"pbs{b}_{w0}"
                    )
                    nc.sync.dma_start(
                        t[:, 0:w1 - w0, :],
                        rdram[b, 2 * w0:2 * w1, 0:CW]
                        .partition_broadcast(128),
                    )
                    return t

                def emit_evict(ts, pc, pb, loc):
                    nt = min(TS, TOUT - ts * TS)
                    # DVE may not apply abs in scalar_tensor_tensor: take
                    # |conv| on ScalarE (PSUM->SBUF), then multiply by
                    # 1/norm + row-sum into acc[:, ts] on DVE
                    scr = scrp.tile(
                        [F, TS], BF16, tag="scr", name=f"scr{b}_{ts}"
                    )
                    nc.scalar.activation(
                        scr[:, 0:nt], pc[:, 0:nt], AF.Abs
                    )
                    nc.vector.scalar_tensor_tensor(
                        scr[:, 0:nt],
                        scr[:, 0:nt],
                        1.0,
                        pb[:, loc, 0:nt],
                        op0=ALU.mult,
                        op1=ALU.mult,
                        accum_out=acc[:, ts:ts + 1],
                    )

                deferred = []
                for wi, (w0, w1) in enumerate(waves or WAVES):
                    if wi in mid:
                        mid[wi]()
                    if deferred:
                        dw0 = deferred[0][0]
                        dpb = emit_pbs(dw0, deferred[-1][0] + 1)
                        for ts, pc in deferred:
                            emit_evict(ts, pc, dpb, ts - dw0)
                        deferred = []
                    wtiles = list(range(w0, w1))
                    pcs = {}
                    for ts in wtiles:
                        pcs[ts] = pconv.tile(
                            [F, TS], F32, name=f"pc_{b}_{ts}", tag="pc"
                        )
                    # prefetch 1/norm broadcast rows while the wave runs
                    pbs = None
                    if wi not in late_pbs:
                        pbs = emit_pbs(w0, w1)
                    # kq-outer: one stationary serves len(wtiles) matmuls;
                    # followers skip the redundant LDWEIGHTS
                    for kq in range(KQ):
                        for i, ts in enumerate(wtiles):
                            t0 = ts * TS
                            nt = min(TS, TOUT - t0)
                            nt_mm = nt + (nt & 1)
                            mvt, base = (
                                xf8map(ts) if xf8map is not None else (xf8, 0)
                            )
                            u0 = t0 - base + 2 * kq
                            bi = nc.tensor.matmul(
                                pcs[ts][:, 0:nt_mm],
                                wtile[:, kq, :],
                                mvt[:, 0:2, u0:u0 + nt_mm],
                                start=(kq == 0),
                                stop=(kq == KQ - 1),
                                perf_mode=DR,
                            )
                            if i > 0:
                                bi.ins.ldweights = False
                    if wi in late_pbs:
                        deferred = [(ts, pcs[ts]) for ts in wtiles]
                    else:
                        for ts in wtiles:
                            emit_evict(ts, pcs[ts], pbs, ts - w0)
                if deferred:
                    dw0 = deferred[0][0]
                    dpb = emit_pbs(dw0, deferred[-1][0] + 1)
                    for ts, pc in deferred:
                        emit_evict(ts, pc, dpb, ts - dw0)
                nc.vector.reduce_sum(
                    S[:, b:b + 1], acc[:], axis=mybir.AxisListType.X
                )

            # ---------------- pair 0: chunked cold start --------------------
            # x arrives in three chunks; batch-A's moving operand is split
            # into L/M/R tiles so conv wave 0 (tiles 0-1) waits only on
            # the four L casts built from the small first chunk: the
            # framework's counting-semaphore deps would otherwise make
            # wave 0 wait on every writer of a single whole-T tile.
            WL, WM, MBASE, WR, RBASE = 1088, 1600, 1024, 1536, 2560
            C0, C1 = 1152, 2688
            xt2_0 = xtp.tile([128, T], F32, tag="xt", name="xt0c")
            nc.sync.dma_start(xt2_0[:, 0:C0], xs_in[0:2, :, 0:C0])
            nc.sync.dma_start(xt2_0[:, C0:C1], xs_in[0:2, :, C0:C1])
            nc.sync.dma_start(xt2_0[:, C1:T], xs_in[0:2, :, C1:T])
            xsq8_0 = sqp.tile([128, 2, T2], F8, tag="xsq", name="xsq0")
            xf8_l = xf8sp.tile([128, 2, WL], F8, tag="xf8s", name="xf8L")
            cast_planes(xf8_l, xt2_0, 0, 0, WL, 0, WL)
            xf8_m = xf8sp.tile([128, 2, WM], F8, tag="xf8s", name="xf8M")
            cast_planes(xf8_m, xt2_0, 0, MBASE, WM, 0, WM)
            square_op(xsq8_0, xt2_0, 0, 0, T2)
            xf8_r = new_xf8(0, 0, W=WR, tag="xf8s", pool=xf8sp)
            cast_planes(xf8_r, xt2_0, 0, RBASE, WR, 0, WR)
            square_op(xsq8_0, xt2_0, 1, 0, T2)

            def map0(ts):
                if ts < 2:
                    return (xf8_l, 0)
                if ts < 5:
                    return (xf8_m, MBASE)
                return (xf8_r, RBASE)

            xf8_b = new_xf8(0, 1)

            st = {"p": 0, "xt2": None, "xsq8": None, "s4": None,
                  "a": None, "b": None}

            def prep0_mid():
                # pair 0's norm chain + batch-B casts, emitted between
                # batch 0's conv waves (ScalarE/DVE/PE all idle there)
                s4 = sq_group(0, xsq8_0)
                cast_half(xf8_b, xt2_0, 1, 0, T)
                norm_pair(0, s4)

            def mk_loadx(q):
                def f():
                    st["xt2"] = load_x(q)
                return f

            def mk_casta(q):
                def f():
                    st["a"] = new_xf8(q, 0)
                    cast_half(st["a"], st["xt2"], 0, 0, T)
                return f

            def mk_square(q):
                def f():
                    st["xsq8"] = sqp.tile(
                        [128, 2, T2], F8, tag="xsq", name=f"xsq{q}"
                    )
                    square_op(st["xsq8"], st["xt2"], 0, 0, T2)
                    square_op(st["xsq8"], st["xt2"], 1, 0, T2)
                return f

            def mk_sqnorm(q):
                def f():
                    s4 = sq_group(q, st["xsq8"])
                    norm_pair(q, s4)
                return f

            def mk_castb(q):
                def f():
                    st["b"] = new_xf8(q, 1)
                    cast_half(st["b"], st["xt2"], 1, 0, T)
                return f

            # pair 0 convs (prep for pair 1 embedded); batch 0's wave-0
            # pbs DMA must be emitted after prep0_mid's rdram write.
            # load_x(1) sits at mid[1] so its 2 MB doesn't compete with
            # pair 0's halves for DMA bandwidth.
            conv_batch(
                0, None, wdr, xf8map=map0, waves=[(0, 2), (2, 5), (5, 8)],
                mid={1: lambda: (prep0_mid(), mk_loadx(1)())}, late_pbs=(0,),
            )
            conv_batch(
                1, xf8_b, wdr_o,
                mid={1: lambda: (mk_square(1)(), mk_casta(1)()),
                     2: lambda: (mk_sqnorm(1)(), mk_castb(1)())},
            )

            # steady pairs 1..3 (prep for pair q=p+1 embedded)
            for p in range(1, 4):
                xf8_a, xf8_b = st["a"], st["b"]
                q = p + 1
                if q < 4:
                    mid_a = {0: mk_loadx(q), 1: mk_casta(q), 2: mk_square(q)}
                    mid_b = {1: mk_sqnorm(q), 2: mk_castb(q)}
                else:
                    mid_a = None
                    mid_b = None
                conv_batch(2 * p, xf8_a, wdr, mid=mid_a)
                conv_batch(2 * p + 1, xf8_b, wdr_o, mid=mid_b)

            # ---- finish: out[b] = sum_f g[f]*S[f,b] + sum_f bias[f] ------
            pf = pconv.tile([1, BLOC], F32, name="pf", tag="pc")
            nc.tensor.matmul(pf[:], gsb[:], S[:])
            out_sb = constp.tile([1, BLOC], F32)
            nc.scalar.add(out_sb[:], pf[:], bsumsb[0:1, 0:1])
            nc.sync.dma_start(out_d[:], out_sb[:])

    ndup = dedupe_ldweights(nc)
    print(f"dedupe_ldweights: replaced {ndup}", file=sys.stderr)
    return nc


_PROGRAM: bass.Bass | None = None


def _get_program() -> bass.Bass:
    global _PROGRAM
    if _PROGRAM is None:
        _PROGRAM = build_program()
    return _PROGRAM


# ---------------------------------------------------------------------------
# Host entry point
# ---------------------------------------------------------------------------

def host_params(conv_weights, spat_weights, weight, bias):
    """Tiny host-side precomputation of stationaries and scalars."""
    conv = np.asarray(conv_weights, dtype=np.float64)
    spat = np.asarray(spat_weights, dtype=np.float64)
    w = np.asarray(weight, dtype=np.float64)
    bb = np.asarray(bias, dtype=np.float64)

    # prod[k, c, f] = conv[f, k] * spat[f, c]; DoubleRow packing
    # k = 32*j + 2*kq + k2  ->  wdr[(k2,c), kq, j, f]
    prod = np.einsum("fk,fc->kcf", conv, spat) * SW    # [K, C, F]
    P5 = prod.reshape(2, KQ, 2, CIN, F)                # [j, kq, k2, c, f]
    wdr = np.ascontiguousarray(P5.transpose(2, 3, 1, 0, 4)).reshape(
        128, KQ, 2, F
    )
    # SwInterleave layout: stored[p, kq, 2*(127-f) + j] = W[p, kq, j, f]
    wdr = np.ascontiguousarray(
        wdr[:, :, :, ::-1].transpose(0, 1, 3, 2)
    ).reshape(128, KQ, 2 * F)
    wdr = np.clip(wdr, -240.0, 240.0)
    f8np = mybir.dt.np(F8)
    wdr = wdr.astype(np.float32)
    wdro = np.roll(wdr, 64, axis=0)
    wdr = wdr.astype(f8np)
    wdro = wdro.astype(f8np)

    # block-diag ones stationary for the norm c-sums: output row
    # f = 2h + j sums plane j over batch-half h's partitions
    W5 = np.zeros((128, 2, F), np.float32)             # [p, j, f]
    W5[0:64, 0, 0] = 1.0
    W5[0:64, 1, 1] = 1.0
    W5[64:128, 0, 2] = 1.0
    W5[64:128, 1, 3] = 1.0
    ones8 = np.ascontiguousarray(
        W5[:, :, ::-1].transpose(0, 2, 1)
    ).reshape(128, 2 * F).astype(f8np)

    norm_w = np.sqrt((spat * spat).sum(1) * (conv * conv).sum(1))  # [F]
    g = (SCALE * SQS / (TOUT * norm_w * SW * SX) * w).astype(
        np.float32
    ).reshape(F, 1)
    bsum = np.array([[bb.sum()]], dtype=np.float32)
    return wdr, wdro, ones8, g, bsum


def make_in_maps(x, conv_weights, spat_weights, weight, bias):
    x = np.ascontiguousarray(np.asarray(x, dtype=np.float32))
    wdr, wdro, ones8, g, bsum = host_params(
        conv_weights, spat_weights, weight, bias
    )
    in_maps = []
    for c in range(NCORES):
        in_maps.append(
            {
                "xs": np.ascontiguousarray(x[c * BLOC:(c + 1) * BLOC]),
                "wdr": wdr,
                "wdro": wdro,
                "g": g,
                "bsum": bsum,
                "ones8": ones8,
            }
        )
    return in_maps


def kernel(x, conv_weights, spat_weights, weight, bias):
    in_maps = make_in_maps(x, conv_weights, spat_weights, weight, bias)
    nc = _get_program()
    res = run_bass_kernel_spmd(nc, in_maps, core_ids=list(range(NCORES)))
    out = np.concatenate(
        [res.results[c]["out"].reshape(BLOC) for c in range(NCORES)]
    )
    return out.astype(np.float32)
